# revision 7
# baseline (speedup 1.0000x reference)
"""Trainium2 Bass kernel for the OOTG SetConv (Gaussian-kernel message passing).

Computes: out[m,g,e] = z_grid[m,g,e] + sum_n exp(-0.5*||xg'[m,g]-x'[m,n]||^2) * z[m,n,e]
where primed coords are divided by the per-dim lengthscale.

Per core (8 cores, grid axis g sharded 16384 -> 8 x 2048):
  - S[n,g] = x'.xg' - 0.5||xg'||^2 - 0.5||x'||^2 as ONE K=12 matmul: the
    coordinates and norm terms ride in as contraction rows, split bf16
    hi/lo with all four cross products per dim, so the matmul streams at
    1 col/cycle while keeping |dS| ~ 1e-3.
  - E = exp(S) on ScalarE straight out of PSUM, written as bf16 (the
    throughput bottleneck: m*g*n/8 = 16.8M exps/core at 1 elem/lane/cycle;
    one ACTIVATE per [128, 1536] three-bank PSUM group).
  - out[e,g] += z[n,e].T @ E[n,g] over n-tiles, with z as bf16 hi/lo
    pairs in the stationary operand (hi half -> PSUM partitions 0-63,
    lo half -> 64-127; summed during evacuation) so z keeps ~fp32
    precision at zero extra stream cost.
  - The result stays in [e, g] layout: the hi+lo+z_grid combine happens
    on DVE as two adds, the [g, e] transpose is pure layout and is done
    on host during unpack (no PE transposes, no identity matrix).
The loop is software-pipelined (the mm2 of group q-1 issues before the
exp of group q) so the PE never waits on ScalarE. Input DMAs are staged
fine-grained (per-m xT/zr, per-block gT) in dependency order so the
first matmul starts ~3us in, not after the full 6MB input load.
All host-side work is O((n+g)*dx) layout/prep plus the final pure-layout
transpose; the heavy math runs on device.
"""

import sys
from collections import deque

import numpy as np

try:
    import concourse.bass as bass
except ImportError:
    sys.path.insert(0, "/opt/trn_rl_repo")
    import concourse.bass as bass

import concourse.bacc as bacc
import concourse.mybir as mybir
import concourse.tile as tile
from concourse.bass_utils import run_bass_kernel_spmd

try:
    import ml_dtypes

    BF16_NP = ml_dtypes.bfloat16
except ImportError:  # pragma: no cover
    BF16_NP = None

N_CORES = 8
M, N, DX, DZ, H, W = 2, 4096, 2, 64, 128, 128
G = H * W                 # 16384 grid points (flattened)
GC = G // N_CORES         # 2048 grid rows per core per batch
NT = N // 128             # 32 n-tiles of 128
NGI = 4                   # g sub-chunks per batch per core
GCH = GC // NGI           # 512 g columns per sub-chunk
KT = GCH // 128           # 4 g-tiles of 128 rows per sub-chunk
E = DZ                    # 64
KC = 12                   # contraction rows of the S matmul (bf16 hi/lo)
NBLK = M * NGI            # 8 (m, gi) blocks per core
F32 = mybir.dt.float32
BF16 = mybir.dt.bfloat16

# n-tile groups per (m, gi) block: ten triples + one pair = 32 tiles.
GROUPS = [(3 * q, 3) for q in range(10)] + [(30, 2)]
# block 0 ramps in with 1- and 2-tile groups so the first EXP only waits
# on a single matmul issued on the still-cold PE (one extra ACT commit,
# ~0.4us, buys ~2us of pipeline-fill)
GROUPS0 = [(0, 1), (1, 2)] + [(3 + 3 * q, 3) for q in range(9)] + [(30, 2)]


def build_nc():
    nc = bacc.Bacc(None, target_bir_lowering=False)
    # K padded from KC=12 to 128 with zero rows: the PE streams moving
    # operands at half rate when the contraction dim is <= 64, and
    # small-K matmuls also slow down neighboring full-K ones
    xT_d = nc.dram_tensor("xT", [128, M * N], BF16, kind="ExternalInput")
    gT_d = nc.dram_tensor("gT", [128, M * GC], BF16, kind="ExternalInput")
    zr_d = nc.dram_tensor("zr", [128, M * NT * 128], BF16, kind="ExternalInput")
    zgr_d = nc.dram_tensor("zgr", [E, M * GC], F32, kind="ExternalInput")
    out_d = nc.dram_tensor("out", [E, M * GC], F32, kind="ExternalOutput")
    act_exp = mybir.ActivationFunctionType.Exp

    with tile.TileContext(nc) as tc:
        with (
            tc.tile_pool(name="consts", bufs=1) as consts,
            tc.tile_pool(name="zg", bufs=4) as zgp,
            tc.tile_pool(name="epool", bufs=6) as epool,
            tc.tile_pool(name="opool", bufs=2) as opool,
            tc.tile_pool(name="fpool", bufs=2) as fpool,
            tc.tile_pool(name="ps_s", bufs=2, space=bass.MemorySpace.PSUM) as ps_s,
            tc.tile_pool(name="ps_o", bufs=2, space=bass.MemorySpace.PSUM) as ps_o,
        ):
            # Input staging across BOTH hwdge queues (Sync + Scalar; the
            # Scalar queue is idle until the first real EXP, and launches
            # emitted before the warm ACTIVATE run immediately). Ordered
            # by first use with per-slice tiles so the first matmul fires
            # as soon as its own data lands instead of after the whole
            # 6MB input load. The warm exp keeps the ~1.3us ACT table
            # load overlapped with the input DMAs.
            xT_sb = []
            for m in range(M):
                t_ = consts.tile([128, N], BF16, tag=f"xt{m}", name=f"xt{m}")
                xT_sb.append(t_)
            gT_sb = []
            for blk in range(NBLK):
                t_ = consts.tile([128, GCH], BF16, tag=f"gt{blk}", name=f"gt{blk}")
                gT_sb.append(t_)
            zr_sb = []
            for m in range(M):
                t_ = consts.tile([128, NT * 128], BF16, tag=f"zr{m}", name=f"zr{m}")
                zr_sb.append(t_)

            warm = consts.tile([1, 8], F32)
            nc.gpsimd.memset(warm[:], 0.0)
            # scalar-queue: one tiny critical chunk, then the table load +
            # warm exp, then the rest of xT[m=0] (launches after the warm
            # ACTIVATE still overlap the cold-PE matmuls of block 0)
            nc.scalar.dma_start(xT_sb[0][:, 0:512], xT_d[:, 0:512])
            nc.scalar.activation(warm[:], warm[:], act_exp)
            nc.scalar.dma_start(xT_sb[0][:, 512:1024], xT_d[:, 512:1024])
            nc.scalar.dma_start(xT_sb[0][:, 1024:2048], xT_d[:, 1024:2048])
            nc.scalar.dma_start(xT_sb[0][:, 2048:N], xT_d[:, 2048:N])
            # sync-queue launches, first-needed first
            nc.sync.dma_start(gT_sb[0][:], gT_d[:, 0:GCH])
            nc.sync.dma_start(zr_sb[0][:, 0:512], zr_d[:, 0:512])
            nc.sync.dma_start(zr_sb[0][:, 512:2048], zr_d[:, 512:2048])
            nc.sync.dma_start(gT_sb[1][:], gT_d[:, GCH : 2 * GCH])
            nc.sync.dma_start(
                zr_sb[0][:, 2048 : NT * 128], zr_d[:, 2048 : NT * 128]
            )
            zg0 = zgp.tile([E, GCH], F32, name="zg0")
            nc.sync.dma_start(zg0[:], zgr_d[:, 0:GCH])
            nc.sync.dma_start(xT_sb[1][:], xT_d[:, N : 2 * N])
            nc.sync.dma_start(zr_sb[1][:], zr_d[:, NT * 128 : 2 * NT * 128])
            # gT for blocks 2..7 is emitted inside the loop (one per early
            # step) so later input DMAs don't delay block 0/1 traffic
            late_gt = deque(range(2, NBLK))

            # Flat software pipeline over all (block, n-tile-group) steps:
            #   step i emits: mm1(i) -> mm2(i-2) -> exp(i)
            # The two-group lag on mm2 keeps the matmul feeding exp(i)
            # well clear of exp(i-1)'s end, so ScalarE (the bottleneck)
            # runs back to back. Block epilogues are split: the DVE
            # combine issues right after the block's last mm2 (two adds:
            # psum-hi + z_grid, then + psum-lo), then the DMA out.
            state = {}
            q2 = deque()
            todo = deque()

            def make_mm2(e_g, t0, cnt, m, blk):
                def emit(cur_idx):
                    o_ps = state[blk]["o_ps"]
                    for i in range(cnt):
                        t = t0 + i
                        base = (m * NT + t) * 128
                        nc.tensor.matmul(
                            o_ps[:, :],
                            zr_sb[m][:, (t * 128) : (t * 128) + 128],
                            e_g[:, i * GCH : (i + 1) * GCH],
                            start=(t == 0),
                            stop=(t == NT - 1),
                        )
                    if t0 + cnt == NT:
                        # block finished: combine on DVE 2 steps later
                        todo.append((cur_idx + 2, make_tail(blk)))

                return emit

            def make_tail(blk):
                def emit(cur_idx):
                    o_ps = state[blk]["o_ps"]
                    zg_t = state[blk]["zg_t"]
                    o_half = opool.tile([E, GCH], F32, tag="oh")
                    nc.vector.tensor_add(o_half[:], o_ps[0:E, :], zg_t[:])
                    fin = fpool.tile([E, GCH], F32, tag="fin")
                    nc.vector.tensor_add(fin[:], o_half[:], o_ps[E : 2 * E, :])
                    nc.sync.dma_start(
                        out_d[:, blk * GCH : (blk + 1) * GCH], fin[:]
                    )

                return emit

            seq = [
                (m, gi, t0, cnt)
                for m in range(M)
                for gi in range(NGI)
                for (t0, cnt) in (GROUPS0 if m == 0 and gi == 0 else GROUPS)
            ]
            nseq = len(seq)
            for idx, (m, gi, t0, cnt) in enumerate(seq):
                blk = m * NGI + gi
                if t0 == 0:
                    if blk == 0:
                        zg_t = zg0
                    else:
                        zg_t = zgp.tile([E, GCH], F32)
                        nc.sync.dma_start(
                            zg_t[:], zgr_d[:, blk * GCH : (blk + 1) * GCH]
                        )
                    o_ps = ps_o.tile([128, GCH], F32)
                    state[blk] = {"o_ps": o_ps, "zg_t": zg_t}
                if idx >= 1 and late_gt:
                    b_ = late_gt.popleft()
                    nc.sync.dma_start(
                        gT_sb[b_][:], gT_d[:, b_ * GCH : (b_ + 1) * GCH]
                    )
                s_g = ps_s.tile([128, cnt * GCH], F32, tag="sg")
                for i in range(cnt):
                    t = t0 + i
                    nc.tensor.matmul(
                        s_g[:, i * GCH : (i + 1) * GCH],
                        xT_sb[m][:, t * 128 : (t + 1) * 128],
                        gT_sb[blk][:, :],
                        start=True,
                        stop=True,
                    )
                # drop to lag-1 for the final two steps: there is no
                # later exp for the flushed mm2s to hide behind, so a
                # shorter post-exp backlog shortens the kernel tail
                lag = 1 if idx >= nseq - 2 else 2
                while len(q2) >= lag:
                    q2.popleft()(idx)
                while todo and todo[0][0] <= idx:
                    todo.popleft()[1](idx)
                e_g = epool.tile([128, cnt * GCH], BF16, tag="eg")
                nc.scalar.activation(e_g[:], s_g[:], act_exp)
                q2.append(make_mm2(e_g, t0, cnt, m, blk))
            nidx = len(seq)
            while q2:
                q2.popleft()(nidx)
            while todo:
                todo.popleft()[1](nidx)
    nc.compile()
    return nc


def _split_bf16(a):
    hi = a.astype(BF16_NP)
    lo = (a - hi.astype(np.float32)).astype(BF16_NP)
    return hi, lo


def prep_inputs(x, z, x_grid, z_grid, lengthscale_param):
    """Host-side layout prep + sharding. Returns per-core input maps."""
    x = np.asarray(x, dtype=np.float32)
    z = np.asarray(z, dtype=np.float32)
    x_grid = np.asarray(x_grid, dtype=np.float32)
    z_grid = np.asarray(z_grid, dtype=np.float32)
    p = np.asarray(lengthscale_param, dtype=np.float32)

    ls = (np.float32(1e-5) + np.logaddexp(p, np.float32(0.0))).astype(np.float32)
    xs = (x / ls).astype(np.float32)                      # [M, N, DX]
    xg = (x_grid.reshape(M, G, DX) / ls).astype(np.float32)

    xnorm = (-0.5 * (xs[..., 0] * xs[..., 0] + xs[..., 1] * xs[..., 1])).astype(
        np.float32
    )
    gnorm = (-0.5 * (xg[..., 0] * xg[..., 0] + xg[..., 1] * xg[..., 1])).astype(
        np.float32
    )
    # bf16 hi/lo split of every operand; S = sum_d x_d*g_d + xn*1 + 1*gn
    # with each product fully expanded: (xh+xl)*(gh+gl) -> 4 rows per dim
    xh0, xl0 = _split_bf16(xs[..., 0])
    xh1, xl1 = _split_bf16(xs[..., 1])
    gh0, gl0 = _split_bf16(xg[..., 0])
    gh1, gl1 = _split_bf16(xg[..., 1])
    xnh, xnl = _split_bf16(xnorm)
    gnh, gnl = _split_bf16(gnorm)
    on = np.ones((M, N), BF16_NP)
    og = np.ones((M, G), BF16_NP)

    xT = np.zeros((128, M * N), BF16_NP)
    xT[:KC] = np.stack(
        [xh0, xh0, xl0, xl0, xh1, xh1, xl1, xl1, xnh, xnl, on, on], axis=0
    ).reshape(KC, M * N)
    gT_full = np.stack(
        [gh0, gl0, gh0, gl0, gh1, gl1, gh1, gl1, og, og, gnh, gnl], axis=0
    )  # [KC, M, G]
    zh, zl = _split_bf16(z)                               # [M, N, E] each
    zr = np.ascontiguousarray(
        np.concatenate([zh.reshape(M, NT, 128, E), zl.reshape(M, NT, 128, E)], axis=3)
        .transpose(2, 0, 1, 3)
        .reshape(128, M * NT * 128)
    )
    zg_full = z_grid.reshape(M, G, E)

    in_maps = []
    for c in range(N_CORES):
        sl = slice(c * GC, (c + 1) * GC)
        gT = np.zeros((128, M * GC), BF16_NP)
        gT[:KC] = gT_full[:, :, sl].reshape(KC, M * GC)
        zgr = np.ascontiguousarray(
            zg_full[:, sl].transpose(2, 0, 1).reshape(E, M * GC)
        )
        in_maps.append({"xT": xT, "gT": gT, "zr": zr, "zgr": zgr})
    return in_maps


def unpack_outputs(results):
    outs = []
    for c in range(N_CORES):
        o = np.asarray(results[c]["out"])                 # [E, M*GC]
        outs.append(o.reshape(E, M, GC).transpose(1, 2, 0))  # [M, GC, E]
    full = np.concatenate(outs, axis=1)          # [M, G, E]
    return full.reshape(M, H, W, E).astype(np.float32)


def kernel(x, z, x_grid, z_grid, lengthscale_param):
    in_maps = prep_inputs(x, z, x_grid, z_grid, lengthscale_param)
    nc = build_nc()
    res = run_bass_kernel_spmd(nc, in_maps, list(range(N_CORES)))
    return unpack_outputs(res.results)


# revision 10
# speedup vs baseline: 1.0116x; 1.0116x over previous
"""Trainium2 Bass kernel for the OOTG SetConv (Gaussian-kernel message passing).

Computes: out[m,g,e] = z_grid[m,g,e] + sum_n exp(-0.5*||xg'[m,g]-x'[m,n]||^2) * z[m,n,e]
where primed coords are divided by the per-dim lengthscale.

Per core (8 cores, grid axis g sharded 16384 -> 8 x 2048):
  - S[n,g] = x'.xg' - 0.5||xg'||^2 - 0.5||x'||^2 as ONE K=12 matmul: the
    coordinates and norm terms ride in as contraction rows, split bf16
    hi/lo with all four cross products per dim, so the matmul streams at
    1 col/cycle while keeping |dS| ~ 1e-3.
  - E = exp(S) on ScalarE straight out of PSUM, written as bf16 (the
    throughput bottleneck: m*g*n/8 = 16.8M exps/core at 1 elem/lane/cycle;
    one ACTIVATE per [128, 1536] three-bank PSUM group).
  - out[e,g] += z[n,e].T @ E[n,g] over n-tiles, with z as bf16 hi/lo
    pairs in the stationary operand (hi half -> PSUM partitions 0-63,
    lo half -> 64-127; summed during evacuation) so z keeps ~fp32
    precision at zero extra stream cost.
  - The result stays in [e, g] layout: the hi+lo+z_grid combine happens
    on DVE as two adds, the [g, e] transpose is pure layout and is done
    on host during unpack (no PE transposes, no identity matrix).
The loop is software-pipelined (the mm2 of group q-1 issues before the
exp of group q) so the PE never waits on ScalarE. Input DMAs are staged
fine-grained (per-m xT/zr, per-block gT) in dependency order so the
first matmul starts ~3us in, not after the full 6MB input load.
All host-side work is O((n+g)*dx) layout/prep plus the final pure-layout
transpose; the heavy math runs on device.
"""

import sys
from collections import deque

import numpy as np

try:
    import concourse.bass as bass
except ImportError:
    sys.path.insert(0, "/opt/trn_rl_repo")
    import concourse.bass as bass

import concourse.bacc as bacc
import concourse.mybir as mybir
import concourse.tile as tile
from concourse.bass_utils import run_bass_kernel_spmd

try:
    import ml_dtypes

    BF16_NP = ml_dtypes.bfloat16
except ImportError:  # pragma: no cover
    BF16_NP = None

N_CORES = 8
M, N, DX, DZ, H, W = 2, 4096, 2, 64, 128, 128
G = H * W                 # 16384 grid points (flattened)
GC = G // N_CORES         # 2048 grid rows per core per batch
NT = N // 128             # 32 n-tiles of 128
NGI = 4                   # g sub-chunks per batch per core
GCH = GC // NGI           # 512 g columns per sub-chunk
KT = GCH // 128           # 4 g-tiles of 128 rows per sub-chunk
E = DZ                    # 64
KC = 12                   # contraction rows of the S matmul (bf16 hi/lo)
NBLK = M * NGI            # 8 (m, gi) blocks per core
F32 = mybir.dt.float32
BF16 = mybir.dt.bfloat16

# n-tile groups per (m, gi) block: ten triples + one pair = 32 tiles.
GROUPS = [(3 * q, 3) for q in range(10)] + [(30, 2)]


def build_nc():
    nc = bacc.Bacc(None, target_bir_lowering=False)
    # K padded from KC=12 to 128 with zero rows: the PE streams moving
    # operands at half rate when the contraction dim is <= 64, and
    # small-K matmuls also slow down neighboring full-K ones
    xT_d = nc.dram_tensor("xT", [128, M * N], BF16, kind="ExternalInput")
    gT_d = nc.dram_tensor("gT", [128, M * GC], BF16, kind="ExternalInput")
    zr_d = nc.dram_tensor("zr", [128, M * NT * 128], BF16, kind="ExternalInput")
    zgr_d = nc.dram_tensor("zgr", [E, M * GC], F32, kind="ExternalInput")
    out_d = nc.dram_tensor("out", [E, M * GC], F32, kind="ExternalOutput")
    act_exp = mybir.ActivationFunctionType.Exp

    with tile.TileContext(nc) as tc:
        with (
            tc.tile_pool(name="consts", bufs=1) as consts,
            tc.tile_pool(name="zg", bufs=4) as zgp,
            tc.tile_pool(name="epool", bufs=6) as epool,
            tc.tile_pool(name="opool", bufs=2) as opool,
            tc.tile_pool(name="fpool", bufs=2) as fpool,
            tc.tile_pool(name="ps_s", bufs=2, space=bass.MemorySpace.PSUM) as ps_s,
            tc.tile_pool(name="ps_o", bufs=2, space=bass.MemorySpace.PSUM) as ps_o,
        ):
            # Input staging across BOTH hwdge queues (Sync + Scalar; the
            # Scalar queue is idle until the first real EXP, and launches
            # emitted before the warm ACTIVATE run immediately). Ordered
            # by first use with per-slice tiles so the first matmul fires
            # as soon as its own data lands instead of after the whole
            # 6MB input load. The warm exp keeps the ~1.3us ACT table
            # load overlapped with the input DMAs.
            xT_sb = []
            for m in range(M):
                t_ = consts.tile([128, N], BF16, tag=f"xt{m}", name=f"xt{m}")
                xT_sb.append(t_)
            gT_sb = []
            for blk in range(NBLK):
                t_ = consts.tile([128, GCH], BF16, tag=f"gt{blk}", name=f"gt{blk}")
                gT_sb.append(t_)
            zr_sb = []
            for m in range(M):
                t_ = consts.tile([128, NT * 128], BF16, tag=f"zr{m}", name=f"zr{m}")
                zr_sb.append(t_)

            warm = consts.tile([1, 8], F32)
            nc.gpsimd.memset(warm[:], 0.0)
            # scalar-queue: one critical chunk (enough for block 0's first
            # three groups), then the table load + warm exp; everything
            # else rides the sync queue so no launch sits between the
            # warm exp and the first real ACTIVATE on the Scalar engine
            nc.scalar.dma_start(xT_sb[0][:, 0:1024], xT_d[:, 0:1024])
            nc.scalar.activation(warm[:], warm[:], act_exp)
            # sync-queue launches, first-needed first
            nc.sync.dma_start(gT_sb[0][:], gT_d[:, 0:GCH])
            nc.sync.dma_start(xT_sb[0][:, 1024:2048], xT_d[:, 1024:2048])
            nc.sync.dma_start(zr_sb[0][:, 0:1024], zr_d[:, 0:1024])
            nc.sync.dma_start(xT_sb[0][:, 2048:N], xT_d[:, 2048:N])
            nc.sync.dma_start(gT_sb[1][:], gT_d[:, GCH : 2 * GCH])
            nc.sync.dma_start(
                zr_sb[0][:, 1024 : NT * 128], zr_d[:, 1024 : NT * 128]
            )
            zg0 = zgp.tile([E, GCH], F32, name="zg0")
            nc.sync.dma_start(zg0[:], zgr_d[:, 0:GCH])
            nc.sync.dma_start(xT_sb[1][:], xT_d[:, N : 2 * N])
            nc.sync.dma_start(zr_sb[1][:], zr_d[:, NT * 128 : 2 * NT * 128])
            # gT for blocks 2..7 is emitted inside the loop (one per early
            # step) so later input DMAs don't delay block 0/1 traffic
            late_gt = deque(range(2, NBLK))

            # Flat software pipeline over all (block, n-tile-group) steps:
            #   step i emits: mm1(i) -> mm2(i-2) -> exp(i)
            # The two-group lag on mm2 keeps the matmul feeding exp(i)
            # well clear of exp(i-1)'s end, so ScalarE (the bottleneck)
            # runs back to back. Block epilogues are split: the DVE
            # combine issues right after the block's last mm2 (two adds:
            # psum-hi + z_grid, then + psum-lo), then the DMA out.
            state = {}
            q2 = deque()
            todo = deque()

            def make_mm2(e_g, t0, cnt, m, blk):
                def emit(cur_idx):
                    o_ps = state[blk]["o_ps"]
                    for i in range(cnt):
                        t = t0 + i
                        base = (m * NT + t) * 128
                        nc.tensor.matmul(
                            o_ps[:, :],
                            zr_sb[m][:, (t * 128) : (t * 128) + 128],
                            e_g[:, i * GCH : (i + 1) * GCH],
                            start=(t == 0),
                            stop=(t == NT - 1),
                        )
                    if t0 + cnt == NT:
                        # block finished: combine on DVE 2 steps later
                        todo.append((cur_idx + 2, make_tail(blk)))

                return emit

            def make_tail(blk):
                def emit(cur_idx):
                    o_ps = state[blk]["o_ps"]
                    zg_t = state[blk]["zg_t"]
                    o_half = opool.tile([E, GCH], F32, tag="oh")
                    nc.vector.tensor_add(o_half[:], o_ps[0:E, :], zg_t[:])
                    fin = fpool.tile([E, GCH], F32, tag="fin")
                    nc.vector.tensor_add(fin[:], o_half[:], o_ps[E : 2 * E, :])
                    nc.sync.dma_start(
                        out_d[:, blk * GCH : (blk + 1) * GCH], fin[:]
                    )

                return emit

            seq = [
                (m, gi, t0, cnt)
                for m in range(M)
                for gi in range(NGI)
                for (t0, cnt) in GROUPS
            ]
            nseq = len(seq)
            for idx, (m, gi, t0, cnt) in enumerate(seq):
                blk = m * NGI + gi
                if t0 == 0:
                    if blk == 0:
                        zg_t = zg0
                    else:
                        zg_t = zgp.tile([E, GCH], F32)
                        nc.sync.dma_start(
                            zg_t[:], zgr_d[:, blk * GCH : (blk + 1) * GCH]
                        )
                    o_ps = ps_o.tile([128, GCH], F32)
                    state[blk] = {"o_ps": o_ps, "zg_t": zg_t}
                if idx >= 1 and late_gt:
                    b_ = late_gt.popleft()
                    nc.sync.dma_start(
                        gT_sb[b_][:], gT_d[:, b_ * GCH : (b_ + 1) * GCH]
                    )
                s_g = ps_s.tile([128, cnt * GCH], F32, tag="sg")
                for i in range(cnt):
                    t = t0 + i
                    nc.tensor.matmul(
                        s_g[:, i * GCH : (i + 1) * GCH],
                        xT_sb[m][:, t * 128 : (t + 1) * 128],
                        gT_sb[blk][:, :],
                        start=True,
                        stop=True,
                    )
                # drop to lag-1 for the final two steps: there is no
                # later exp for the flushed mm2s to hide behind, so a
                # shorter post-exp backlog shortens the kernel tail
                lag = 1 if idx >= nseq - 2 else 2
                while len(q2) >= lag:
                    q2.popleft()(idx)
                while todo and todo[0][0] <= idx:
                    todo.popleft()[1](idx)
                e_g = epool.tile([128, cnt * GCH], BF16, tag="eg")
                nc.scalar.activation(e_g[:], s_g[:], act_exp)
                q2.append(make_mm2(e_g, t0, cnt, m, blk))
            nidx = len(seq)
            while q2:
                q2.popleft()(nidx)
            while todo:
                todo.popleft()[1](nidx)
    nc.compile()
    return nc


def _split_bf16(a):
    hi = a.astype(BF16_NP)
    lo = (a - hi.astype(np.float32)).astype(BF16_NP)
    return hi, lo


def prep_inputs(x, z, x_grid, z_grid, lengthscale_param):
    """Host-side layout prep + sharding. Returns per-core input maps."""
    x = np.asarray(x, dtype=np.float32)
    z = np.asarray(z, dtype=np.float32)
    x_grid = np.asarray(x_grid, dtype=np.float32)
    z_grid = np.asarray(z_grid, dtype=np.float32)
    p = np.asarray(lengthscale_param, dtype=np.float32)

    ls = (np.float32(1e-5) + np.logaddexp(p, np.float32(0.0))).astype(np.float32)
    xs = (x / ls).astype(np.float32)                      # [M, N, DX]
    xg = (x_grid.reshape(M, G, DX) / ls).astype(np.float32)

    xnorm = (-0.5 * (xs[..., 0] * xs[..., 0] + xs[..., 1] * xs[..., 1])).astype(
        np.float32
    )
    gnorm = (-0.5 * (xg[..., 0] * xg[..., 0] + xg[..., 1] * xg[..., 1])).astype(
        np.float32
    )
    # bf16 hi/lo split of every operand; S = sum_d x_d*g_d + xn*1 + 1*gn
    # with each product fully expanded: (xh+xl)*(gh+gl) -> 4 rows per dim
    xh0, xl0 = _split_bf16(xs[..., 0])
    xh1, xl1 = _split_bf16(xs[..., 1])
    gh0, gl0 = _split_bf16(xg[..., 0])
    gh1, gl1 = _split_bf16(xg[..., 1])
    xnh, xnl = _split_bf16(xnorm)
    gnh, gnl = _split_bf16(gnorm)
    on = np.ones((M, N), BF16_NP)
    og = np.ones((M, G), BF16_NP)

    xT = np.zeros((128, M * N), BF16_NP)
    xT[:KC] = np.stack(
        [xh0, xh0, xl0, xl0, xh1, xh1, xl1, xl1, xnh, xnl, on, on], axis=0
    ).reshape(KC, M * N)
    gT_full = np.stack(
        [gh0, gl0, gh0, gl0, gh1, gl1, gh1, gl1, og, og, gnh, gnl], axis=0
    )  # [KC, M, G]
    zh, zl = _split_bf16(z)                               # [M, N, E] each
    zr = np.ascontiguousarray(
        np.concatenate([zh.reshape(M, NT, 128, E), zl.reshape(M, NT, 128, E)], axis=3)
        .transpose(2, 0, 1, 3)
        .reshape(128, M * NT * 128)
    )
    zg_full = z_grid.reshape(M, G, E)

    in_maps = []
    for c in range(N_CORES):
        sl = slice(c * GC, (c + 1) * GC)
        gT = np.zeros((128, M * GC), BF16_NP)
        gT[:KC] = gT_full[:, :, sl].reshape(KC, M * GC)
        zgr = np.ascontiguousarray(
            zg_full[:, sl].transpose(2, 0, 1).reshape(E, M * GC)
        )
        in_maps.append({"xT": xT, "gT": gT, "zr": zr, "zgr": zgr})
    return in_maps


def unpack_outputs(results):
    outs = []
    for c in range(N_CORES):
        o = np.asarray(results[c]["out"])                 # [E, M*GC]
        outs.append(o.reshape(E, M, GC).transpose(1, 2, 0))  # [M, GC, E]
    full = np.concatenate(outs, axis=1)          # [M, G, E]
    return full.reshape(M, H, W, E).astype(np.float32)


def kernel(x, z, x_grid, z_grid, lengthscale_param):
    in_maps = prep_inputs(x, z, x_grid, z_grid, lengthscale_param)
    nc = build_nc()
    res = run_bass_kernel_spmd(nc, in_maps, list(range(N_CORES)))
    return unpack_outputs(res.results)
